# revision 36
# baseline (speedup 1.0000x reference)
"""Biaffine scorer kernel for Trainium2 (Bass/Tile), data-parallel over batch
across 8 NeuronCores.

Reference computation (per batch item b):
    h = leaky_relu(state @ head_w + head_b)          # (S, BS)
    t = leaky_relu(state @ tail_w + tail_b)          # (S, BS)
    scores1[x,y,o] = h[x] @ U[o] @ t[y]
    scores2[x,y,o] = Wh.h1[x] + Wt.t1[y] + Ww.wemb[x,y] + cls_b
    out = scores1 + scores2                          # (S, S, O)

Full-bf16 dataflow (tolerance 2e-2 rel; bf16 end-to-end measures ~5e-3):
the f32 baseline was DMA-bandwidth-bound, bf16 halves every byte moved.
The batch-independent width/cls_b table C = wproj[pos(x,y), o] is added
on the HOST, so every device-side PSUM evacuation is a single-pass copy,
split across ACT and DVE by a greedy busy-time balancer (GpSimd has no
PSUM port and 2-pass routes double-handle elements). Structure per core
(4 batch items as 2 pairs, 512 moving columns per matmul):

    warmup       ~26 dummy matmuls bridge the framework preamble to the
                 first state tile so the PE HAM throttle grants 2.4 GHz
                 before the projections (they gate the first output byte)
    state DMA    kt-pair quarters on the qAct ring; the kt-major
                 projection matmuls stream right behind the transfers
    ht1[p]       [121, 1024] = Prelu(w.T @ stateT + bias) (h | t), bias
                 and the ones-row feature enter via the ACT bias AP
    tUT[p]       [121, 2, 2560]: per o-pair, [U(o).T | folds] @ t1_bb;
                 contiguous psum, one strided (2,512) evac per group
    out[x,(o,y)] = h1T.T @ tUT in 512-col matmuls, [1024,1024,512]
                 chunks on a 4-deep [128,1024] PSUM pool; every chunk
                 DMAs out the moment its evac lands (qSP ring)

Pairs are software-pipelined: pair-1 proj/tut matmuls fill the PE while
pair-0 finals evacuations drain. The host packs constants, transposes
state, un-interleaves the (o,y)-major bf16 output and adds C in f32.
"""

import numpy as np
import ml_dtypes

import concourse.bass as bass
import concourse.bacc as bacc
import concourse.tile as tile
from concourse import mybir
from concourse.bass_utils import run_bass_kernel_spmd

# problem shape (hardcoded per harness contract)
B, S, H = 32, 255, 1024
BS, WD, O = 120, 20, 10
SP = 256            # padded S
SP2 = 2 * SP        # paired moving dim
NW = SP * O         # 2560
KT = H // 128       # 8
NCORES = 8
BPC = B // NCORES   # 4 batch items per core
NP = BPC // 2       # 2 pairs per core
BSE = BS + 1        # 121

F32 = mybir.dt.float32
BF16 = mybir.dt.bfloat16
NPBF = ml_dtypes.bfloat16

_CACHE: dict = {}


def _emit(tc, d):
    """Emit the per-core program. d: dict of DRAM APs."""
    from contextlib import ExitStack

    nc = tc.nc
    AF = mybir.ActivationFunctionType

    with ExitStack() as ctx:
        const = ctx.enter_context(tc.tile_pool(name="const", bufs=1))
        st_pool = ctx.enter_context(tc.tile_pool(name="st", bufs=2))
        ht_pool = ctx.enter_context(tc.tile_pool(name="ht", bufs=2))
        tut_pool = ctx.enter_context(tc.tile_pool(name="tut", bufs=2))
        out_pool = ctx.enter_context(tc.tile_pool(name="outp", bufs=4))
        # 4-deep [128,1024] PSUM pool: 1024-wide evacs amortize the
        # per-op access latency; depth 4 keeps the PE ahead of the evacs
        pp = ctx.enter_context(tc.tile_pool(name="pp", bufs=4, space="PSUM"))

        # greedy 2-engine evac balancer: C is added on the HOST, so all
        # PSUM evacuations are 1-pass copies split across ACT and DVE.
        busy = {"A": 0.0, "D": 0.0}

        def _act(w):
            return (w + 206) / 1.2

        def _dve(w):
            return (w + 145) / 0.96

        def evac_copy(dst, src, w):
            if busy["A"] + _act(w) <= busy["D"] + _dve(w):
                busy["A"] += _act(w)
                nc.scalar.activation(dst, src, AF.Copy)
            else:
                busy["D"] += _dve(w)
                nc.vector.tensor_copy(dst, src)

        # ---- persistent constants (qSP ring, in first-use order) ----
        # weights carry an extra zero column -> psum row 120 = 0; the ACT
        # bias AP then sets row 120 to Prelu(0 + 1.0) = 1.0 (the ones row).
        sb_hw = const.tile([128, KT * BSE], BF16)
        nc.sync.dma_start(sb_hw[:], d["hw"])
        sb_tw = const.tile([128, KT * BSE], BF16)
        nc.sync.dma_start(sb_tw[:], d["tw"])
        # bias: col 0 = head_b (+1.0 at row 120), col 1 = tail_b
        sb_bias = const.tile([BSE, 2], F32)
        nc.sync.dma_start(sb_bias[:], d["bias"])
        # ut: per-o [121, 121] blocks (U.T with Wt in col 120, Wh folded
        # into the ones-row), then 2 spare cols.
        sb_ut = const.tile([BSE, O * BSE + 2], BF16)
        nc.sync.dma_start(sb_ut[:], d["ut"])
        # ---- state loads (qAct ring): kt-pair quarters so the kt-major
        # projection matmuls stream right behind the DMA ----
        qtr = KT * SP2 // 4
        sb_st = []
        for p in range(NP):
            quarters = []
            for k in range(4):
                t = st_pool.tile([128, qtr], BF16, name=f"sT{k}")
                nc.scalar.dma_start(t[:], d["stateT"][p][:, k * qtr:(k + 1) * qtr])
                quarters.append(t)
            sb_st.append(quarters)

        # ---- PE warmup: dummy matmuls bridge preamble -> first state
        # quarter (~2.5us) so the HAM throttle grants 2.4 GHz before the
        # projections, which gate the first output byte.
        warm = const.tile([128, 128], BF16)
        nc.vector.memset(warm[:], 0.0)
        ps_w = pp.tile([128, 1024], F32, name="ps")
        for i in range(26):
            nc.tensor.matmul(
                ps_w[:, 0:128], lhsT=warm[:], rhs=warm[:], start=True, stop=True
            )

        ht1 = [None, None]   # [121, 1024] bf16: h1T cols 0:512, t1T 512:1024
        tUT = [None, None]   # [121, 2, 2560] bf16 per pair
        for p in range(NP):
            tUT[p] = tut_pool.tile([BSE, 2, NW], BF16, name="tUT")

        def proj(p):
            # head/tail projections -> ht1[p], bf16, via Prelu+bias evac.
            # kt-major order: each state quarter feeds both sides' matmuls
            # as soon as it lands (proj is DMA-paced, not PE-paced).
            ps_ht = pp.tile([128, 1024], F32, name="ps")
            ps_h = ps_ht[0:BSE, 0:512]
            ps_t = ps_ht[0:BSE, 512:1024]
            for kt in range(KT):
                st = sb_st[p][kt // 2]
                rhs = st[:, (kt % 2) * SP2:(kt % 2 + 1) * SP2]
                for ps, w in ((ps_h, sb_hw), (ps_t, sb_tw)):
                    nc.tensor.matmul(
                        ps[:],
                        lhsT=w[:, kt * BSE:(kt + 1) * BSE],
                        rhs=rhs,
                        start=(kt == 0),
                        stop=(kt == KT - 1),
                    )
            ht1[p] = ht_pool.tile([BSE, 2 * SP2], BF16, name="ht1")
            nc.scalar.activation(
                ht1[p][:, 0:SP2], ps_h[:], AF.Prelu,
                bias=sb_bias[:, 0:1], alpha=0.01,
            )
            nc.scalar.activation(
                ht1[p][:, SP2:2 * SP2], ps_t[:], AF.Prelu,
                bias=sb_bias[:, 1:2], alpha=0.01,
            )
            busy["A"] += 2 * _act(512)

        def tut_group(p, q):
            # one 1024-wide PSUM tile = o-pair (2q, 2q+1) for both b0/b1;
            # contiguous psum src, one strided (2,512)-dst evac writes
            # chunk q*512 of both tUT[p][:, bb, :] rows.
            t1T = ht1[p][:, SP2:2 * SP2]
            ps_u = pp.tile([128, 1024], F32, name="ps")[0:BSE]
            for bb in range(2):
                rhs = t1T[:, bb * SP:(bb + 1) * SP]
                for s in range(2):
                    nc.tensor.matmul(
                        ps_u[:, (2 * bb + s) * SP:(2 * bb + s + 1) * SP],
                        lhsT=sb_ut[:, (2 * q + s) * BSE:(2 * q + s + 1) * BSE],
                        rhs=rhs,
                        start=True,
                        stop=True,
                    )
            # contiguous psum src; strided (2,512) dst into both tUT rows
            evac_copy(tUT[p][:, :, q * 512:(q + 1) * 512], ps_u[:], 1024)

        CHUNKS = ((0, 1024), (1024, 1024), (2048, 512))
        sbouts = {}

        def finals_chunk(p, bb, xt, ci):
            # one [128, w] chunk of out[x, (o,y)]; DMAs once its copy lands.
            key = (p, bb, xt)
            if key not in sbouts:
                sbouts[key] = out_pool.tile([128, NW], BF16, name="sb_out")
            sb_out = sbouts[key]
            lo = bb * SP + xt * 128
            lhsT = ht1[p][:, lo:lo + 128]
            c0, w = CHUNKS[ci]
            ps_s = pp.tile([128, 1024], F32, name="ps")
            for s in range(w // 512):
                nc.tensor.matmul(
                    ps_s[:, s * 512:(s + 1) * 512],
                    lhsT=lhsT,
                    rhs=tUT[p][:, bb, c0 + s * 512:c0 + (s + 1) * 512],
                    start=True,
                    stop=True,
                )
            evac_copy(sb_out[:, c0:c0 + w], ps_s[:, 0:w], w)
            nc.sync.dma_start(
                d["out"][2 * p + bb, xt * 128:xt * 128 + 128, c0:c0 + w],
                sb_out[:, c0:c0 + w],
            )

        def finals_tile(p, bb, xt):
            for ci in range(3):
                finals_chunk(p, bb, xt, ci)

        # emit order: first finals tile right after B0; proj(1)/tut(1)
        # matmuls fill PE while pair-0 finals evacs drain.
        proj(0)
        for q in range(5):
            tut_group(0, q)
        finals_tile(0, 0, 0)
        finals_tile(0, 0, 1)
        proj(1)
        finals_tile(0, 1, 0)
        tut_group(1, 0)
        finals_tile(0, 1, 1)
        tut_group(1, 1)
        tut_group(1, 2)
        tut_group(1, 3)
        tut_group(1, 4)
        finals_tile(1, 0, 0)
        finals_tile(1, 0, 1)
        finals_tile(1, 1, 0)
        finals_tile(1, 1, 1)


def build_nc():
    if "nc" in _CACHE:
        return _CACHE["nc"]
    nc = bacc.Bacc(
        "TRN2", target_bir_lowering=False, debug=False, num_devices=NCORES
    )
    d = {}
    d["stateT"] = nc.dram_tensor(
        "stateT", [NP, 128, KT * SP2], BF16, kind="ExternalInput"
    ).ap()
    d["hw"] = nc.dram_tensor("hw", [128, KT * BSE], BF16, kind="ExternalInput").ap()
    d["tw"] = nc.dram_tensor("tw", [128, KT * BSE], BF16, kind="ExternalInput").ap()
    d["ut"] = nc.dram_tensor(
        "ut", [BSE, O * BSE + 2], BF16, kind="ExternalInput"
    ).ap()
    d["bias"] = nc.dram_tensor("bias", [BSE, 2], F32, kind="ExternalInput").ap()
    d["out"] = nc.dram_tensor("out", [BPC, SP, NW], BF16, kind="ExternalOutput").ap()

    with tile.TileContext(nc) as tc:
        _emit(tc, d)
    nc.compile()
    _CACHE["nc"] = nc
    return nc


def prep_inputs(inputs):
    """Host-side constant packing + state transpose. Returns dict of np arrays
    shared across cores (stateT is full-batch; shard before dispatch)."""
    state = np.asarray(inputs["state"], np.float32)
    head_w = np.asarray(inputs["head_w"], np.float32)
    head_b = np.asarray(inputs["head_b"], np.float32)
    tail_w = np.asarray(inputs["tail_w"], np.float32)
    tail_b = np.asarray(inputs["tail_b"], np.float32)
    U = np.asarray(inputs["U"], np.float32)
    cls_w = np.asarray(inputs["cls_w"], np.float32)

    # stateT paired pack: [B/2, 128, (kt, b01, y)], y zero-padded to 256
    stateT = np.zeros((B, H, SP), np.float32)
    stateT[:, :, :S] = state.transpose(0, 2, 1)
    stateT = stateT.reshape(B // 2, 2, KT, 128, SP).transpose(0, 3, 2, 1, 4)
    stateT = np.ascontiguousarray(
        stateT.reshape(B // 2, 128, KT * SP2).astype(NPBF)
    )

    hw_sb = np.zeros((128, KT, BSE), np.float32)
    hw_sb[:, :, :BS] = head_w.reshape(KT, 128, BS).transpose(1, 0, 2)
    hw_sb = np.ascontiguousarray(hw_sb.reshape(128, KT * BSE).astype(NPBF))
    tw_sb = np.zeros((128, KT, BSE), np.float32)
    tw_sb[:, :, :BS] = tail_w.reshape(KT, 128, BS).transpose(1, 0, 2)
    tw_sb = np.ascontiguousarray(tw_sb.reshape(128, KT * BSE).astype(NPBF))

    # ut blocks + 2 spare cols
    ut = np.zeros((BSE, O * BSE + 2), np.float32)
    blocks = ut[:, :O * BSE].reshape(BSE, O, BSE)
    blocks[:BS, :, :BS] = U.transpose(2, 0, 1)           # [j, o, i] = U[o,i,j]
    blocks[:, :, BS] = cls_w[:, BS + 1:2 * (BS + 1)].T   # Wt (incl ones coeff)
    # fold the Wh projection (A-term) into the ones-row of each block:
    # t1T row 120 is all-ones, so adding Wh_ext[o, i] here adds A[x, o]
    # (broadcast over y) to the final scores.
    blocks[BS, :, :] += cls_w[:, :BSE]
    ut = np.ascontiguousarray(ut.astype(NPBF))

    bias = np.zeros((BSE, 2), np.float32)
    bias[:BS, 0] = head_b
    bias[BS, 0] = 1.0                                    # ones-row constant
    bias[:BS, 1] = tail_b
    bias[BS, 1] = 1.0

    return {
        "stateT": stateT,
        "hw": hw_sb,
        "tw": tw_sb,
        "ut": ut,
        "bias": bias,
    }


def run(inputs, trace=False, trace_kwargs=None):
    nc = build_nc()
    full = prep_inputs(inputs)
    shared = {k: v for k, v in full.items() if k != "stateT"}
    in_maps = []
    for c in range(NCORES):
        m = dict(shared)
        m["stateT"] = np.ascontiguousarray(full["stateT"][c * NP:(c + 1) * NP])
        in_maps.append(m)
    res = run_bass_kernel_spmd(
        nc,
        in_maps,
        core_ids=list(range(NCORES)),
        trace=trace,
        **(trace_kwargs or {}),
    )
    out = np.concatenate([r["out"] for r in res.results], axis=0)
    # [B, 256, (o,y)] bf16 -> [B, S, S, O] f32, then + C on the host
    out = out.reshape(B, SP, O, SP).astype(np.float32)
    out = np.ascontiguousarray(out[:, :S, :, :S].transpose(0, 1, 3, 2))
    width_table = np.asarray(inputs["width_table"], np.float32)
    cls_w = np.asarray(inputs["cls_w"], np.float32)
    cls_b = np.asarray(inputs["cls_b"], np.float32)
    pos = np.arange(S)[None, :] - np.arange(S)[:, None] + 1
    pos = pos * (pos > 0)
    wproj = width_table @ cls_w[:, 2 * (BS + 1):].T + cls_b   # [256, 10]
    out += wproj[pos][None]
    return out, res


def kernel(**inputs):
    out, _ = run(inputs, trace=False)
    return out


if __name__ == "__main__":
    build_nc()
    print("build ok")


# revision 38
# speedup vs baseline: 1.0181x; 1.0181x over previous
"""Biaffine scorer kernel for Trainium2 (Bass/Tile), data-parallel over batch
across 8 NeuronCores.

Reference computation (per batch item b):
    h = leaky_relu(state @ head_w + head_b)          # (S, BS)
    t = leaky_relu(state @ tail_w + tail_b)          # (S, BS)
    scores1[x,y,o] = h[x] @ U[o] @ t[y]
    scores2[x,y,o] = Wh.h1[x] + Wt.t1[y] + Ww.wemb[x,y] + cls_b
    out = scores1 + scores2                          # (S, S, O)

Full-bf16 dataflow (tolerance 2e-2 rel; bf16 end-to-end measures ~5e-3):
the f32 baseline was DMA-bandwidth-bound, bf16 halves every byte moved.
The batch-independent width/cls_b table C = wproj[pos(x,y), o] is added
on the HOST, so every device-side PSUM evacuation is a single-pass copy,
split across ACT and DVE by a greedy busy-time balancer (GpSimd has no
PSUM port and 2-pass routes double-handle elements). Structure per core
(4 batch items as 2 pairs, 512 moving columns per matmul):

    warmup       ~26 dummy matmuls bridge the framework preamble to the
                 first state tile so the PE HAM throttle grants 2.4 GHz
                 before the projections (they gate the first output byte)
    state DMA    kt-pair quarters on the qAct ring; the kt-major
                 projection matmuls stream right behind the transfers
    ht1[p]       [121, 1024] = Prelu(w.T @ stateT + bias) (h | t), bias
                 and the ones-row feature enter via the ACT bias AP
    tUT[p]       [121, 2, 2560]: per o-pair, [U(o).T | folds] @ t1_bb;
                 contiguous psum, one strided (2,512) evac per group
    out[x,(o,y)] = h1T.T @ tUT in 512-col matmuls, [1024,1024,512]
                 chunks on a 4-deep [128,1024] PSUM pool; every chunk
                 DMAs out the moment its evac lands (qSP ring)

Pairs are software-pipelined: pair-1 proj/tut matmuls fill the PE while
pair-0 finals evacuations drain. The host packs constants, transposes
state, un-interleaves the (o,y)-major bf16 output and adds C in f32.
"""

import numpy as np
import ml_dtypes

import concourse.bass as bass
import concourse.bacc as bacc
import concourse.tile as tile
from concourse import mybir
from concourse.bass_utils import run_bass_kernel_spmd

# problem shape (hardcoded per harness contract)
B, S, H = 32, 255, 1024
BS, WD, O = 120, 20, 10
SP = 256            # padded S
SP2 = 2 * SP        # paired moving dim
NW = SP * O         # 2560
KT = H // 128       # 8
NCORES = 8
BPC = B // NCORES   # 4 batch items per core
NP = BPC // 2       # 2 pairs per core
BSE = BS + 1        # 121

F32 = mybir.dt.float32
BF16 = mybir.dt.bfloat16
NPBF = ml_dtypes.bfloat16

_CACHE: dict = {}


def _emit(tc, d):
    """Emit the per-core program. d: dict of DRAM APs."""
    from contextlib import ExitStack

    nc = tc.nc
    AF = mybir.ActivationFunctionType

    with ExitStack() as ctx:
        const = ctx.enter_context(tc.tile_pool(name="const", bufs=1))
        st_pool = ctx.enter_context(tc.tile_pool(name="st", bufs=2))
        ht_pool = ctx.enter_context(tc.tile_pool(name="ht", bufs=2))
        tut_pool = ctx.enter_context(tc.tile_pool(name="tut", bufs=2))
        out_pool = ctx.enter_context(tc.tile_pool(name="outp", bufs=4))
        # 4-deep [128,1024] PSUM pool: 1024-wide evacs amortize the
        # per-op access latency; depth 4 keeps the PE ahead of the evacs
        pp = ctx.enter_context(tc.tile_pool(name="pp", bufs=4, space="PSUM"))

        # greedy 2-engine evac balancer: C is added on the HOST, so all
        # PSUM evacuations are 1-pass copies split across ACT and DVE.
        busy = {"A": 0.0, "D": 0.0}

        def _act(w):
            return (w + 206) / 1.2

        def _dve(w):
            return (w + 145) / 0.96

        def evac_copy(dst, src, w):
            if busy["A"] + _act(w) <= busy["D"] + _dve(w):
                busy["A"] += _act(w)
                nc.scalar.activation(dst, src, AF.Copy)
            else:
                busy["D"] += _dve(w)
                nc.vector.tensor_copy(dst, src)

        # ---- persistent constants (qSP ring, in first-use order) ----
        # weights carry an extra zero column -> psum row 120 = 0; the ACT
        # bias AP then sets row 120 to Prelu(0 + 1.0) = 1.0 (the ones row).
        sb_hw = const.tile([128, KT * BSE], BF16)
        nc.sync.dma_start(sb_hw[:], d["hw"])
        sb_tw = const.tile([128, KT * BSE], BF16)
        nc.sync.dma_start(sb_tw[:], d["tw"])
        # ---- state loads: kt-pair quarters so the kt-major projection
        # matmuls stream right behind the DMA. Pair 0 rides the qSP ring
        # directly after the weights (shorter DGE latency than qAct, and
        # qAct's sequencer preamble ends later) -- proj(0) gates the
        # first output byte. Pair 1 rides qAct in parallel.
        qtr = KT * SP2 // 4
        sb_st = []
        for p in range(NP):
            quarters = []
            for k in range(4):
                t = st_pool.tile([128, qtr], BF16, name=f"sT{k}")
                dq = nc.sync if p == 0 else nc.scalar
                dq.dma_start(t[:], d["stateT"][p][:, k * qtr:(k + 1) * qtr])
                quarters.append(t)
            sb_st.append(quarters)
        # bias: col 0 = head_b (+1.0 at row 120), col 1 = tail_b
        sb_bias = const.tile([BSE, 2], F32)
        nc.sync.dma_start(sb_bias[:], d["bias"])
        # ut: per-o [121, 121] blocks (U.T with Wt in col 120, Wh folded
        # into the ones-row), then 2 spare cols.
        sb_ut = const.tile([BSE, O * BSE + 2], BF16)
        nc.sync.dma_start(sb_ut[:], d["ut"])

        # ---- PE warmup: dummy matmuls bridge preamble -> first state
        # quarter (~2.5us) so the HAM throttle grants 2.4 GHz before the
        # projections, which gate the first output byte.
        warm = const.tile([128, 128], BF16)
        nc.vector.memset(warm[:], 0.0)
        ps_w = pp.tile([128, 1024], F32, name="ps")
        for i in range(26):
            nc.tensor.matmul(
                ps_w[:, 0:128], lhsT=warm[:], rhs=warm[:], start=True, stop=True
            )

        ht1 = [None, None]   # [121, 1024] bf16: h1T cols 0:512, t1T 512:1024
        tUT = [None, None]   # [121, 2, 2560] bf16 per pair
        for p in range(NP):
            tUT[p] = tut_pool.tile([BSE, 2, NW], BF16, name="tUT")

        def proj(p):
            # head/tail projections -> ht1[p], bf16, via Prelu+bias evac.
            # kt-major order: each state quarter feeds both sides' matmuls
            # as soon as it lands (proj is DMA-paced, not PE-paced).
            ps_ht = pp.tile([128, 1024], F32, name="ps")
            ps_h = ps_ht[0:BSE, 0:512]
            ps_t = ps_ht[0:BSE, 512:1024]
            for kt in range(KT):
                st = sb_st[p][kt // 2]
                rhs = st[:, (kt % 2) * SP2:(kt % 2 + 1) * SP2]
                for ps, w in ((ps_h, sb_hw), (ps_t, sb_tw)):
                    nc.tensor.matmul(
                        ps[:],
                        lhsT=w[:, kt * BSE:(kt + 1) * BSE],
                        rhs=rhs,
                        start=(kt == 0),
                        stop=(kt == KT - 1),
                    )
            ht1[p] = ht_pool.tile([BSE, 2 * SP2], BF16, name="ht1")
            nc.scalar.activation(
                ht1[p][:, 0:SP2], ps_h[:], AF.Prelu,
                bias=sb_bias[:, 0:1], alpha=0.01,
            )
            nc.scalar.activation(
                ht1[p][:, SP2:2 * SP2], ps_t[:], AF.Prelu,
                bias=sb_bias[:, 1:2], alpha=0.01,
            )
            busy["A"] += 2 * _act(512)

        def tut_group(p, q):
            # one 1024-wide PSUM tile = o-pair (2q, 2q+1) for both b0/b1;
            # contiguous psum src, one strided (2,512)-dst evac writes
            # chunk q*512 of both tUT[p][:, bb, :] rows.
            t1T = ht1[p][:, SP2:2 * SP2]
            ps_u = pp.tile([128, 1024], F32, name="ps")[0:BSE]
            for bb in range(2):
                rhs = t1T[:, bb * SP:(bb + 1) * SP]
                for s in range(2):
                    nc.tensor.matmul(
                        ps_u[:, (2 * bb + s) * SP:(2 * bb + s + 1) * SP],
                        lhsT=sb_ut[:, (2 * q + s) * BSE:(2 * q + s + 1) * BSE],
                        rhs=rhs,
                        start=True,
                        stop=True,
                    )
            # contiguous psum src; strided (2,512) dst into both tUT rows
            evac_copy(tUT[p][:, :, q * 512:(q + 1) * 512], ps_u[:], 1024)

        CHUNKS = ((0, 1024), (1024, 1024), (2048, 512))
        sbouts = {}

        def finals_chunk(p, bb, xt, ci):
            # one [128, w] chunk of out[x, (o,y)]; DMAs once its copy lands.
            key = (p, bb, xt)
            if key not in sbouts:
                sbouts[key] = out_pool.tile([128, NW], BF16, name="sb_out")
            sb_out = sbouts[key]
            lo = bb * SP + xt * 128
            lhsT = ht1[p][:, lo:lo + 128]
            c0, w = CHUNKS[ci]
            ps_s = pp.tile([128, 1024], F32, name="ps")
            for s in range(w // 512):
                nc.tensor.matmul(
                    ps_s[:, s * 512:(s + 1) * 512],
                    lhsT=lhsT,
                    rhs=tUT[p][:, bb, c0 + s * 512:c0 + (s + 1) * 512],
                    start=True,
                    stop=True,
                )
            evac_copy(sb_out[:, c0:c0 + w], ps_s[:, 0:w], w)
            nc.sync.dma_start(
                d["out"][2 * p + bb, xt * 128:xt * 128 + 128, c0:c0 + w],
                sb_out[:, c0:c0 + w],
            )

        def finals_tile(p, bb, xt):
            for ci in range(3):
                finals_chunk(p, bb, xt, ci)

        # emit order: first finals tile right after B0; proj(1)/tut(1)
        # matmuls fill PE while pair-0 finals evacs drain.
        proj(0)
        for q in range(5):
            tut_group(0, q)
        finals_tile(0, 0, 0)
        finals_tile(0, 0, 1)
        proj(1)
        finals_tile(0, 1, 0)
        tut_group(1, 0)
        finals_tile(0, 1, 1)
        tut_group(1, 1)
        tut_group(1, 2)
        tut_group(1, 3)
        tut_group(1, 4)
        finals_tile(1, 0, 0)
        finals_tile(1, 0, 1)
        # interleave the last two tiles chunk-wise so their evacs land on
        # alternating engines and the tail DMAs overlap instead of
        # serializing behind one tile's chain
        for ci in range(3):
            finals_chunk(1, 1, 0, ci)
            finals_chunk(1, 1, 1, ci)


def build_nc():
    if "nc" in _CACHE:
        return _CACHE["nc"]
    nc = bacc.Bacc(
        "TRN2", target_bir_lowering=False, debug=False, num_devices=NCORES
    )
    d = {}
    d["stateT"] = nc.dram_tensor(
        "stateT", [NP, 128, KT * SP2], BF16, kind="ExternalInput"
    ).ap()
    d["hw"] = nc.dram_tensor("hw", [128, KT * BSE], BF16, kind="ExternalInput").ap()
    d["tw"] = nc.dram_tensor("tw", [128, KT * BSE], BF16, kind="ExternalInput").ap()
    d["ut"] = nc.dram_tensor(
        "ut", [BSE, O * BSE + 2], BF16, kind="ExternalInput"
    ).ap()
    d["bias"] = nc.dram_tensor("bias", [BSE, 2], F32, kind="ExternalInput").ap()
    d["out"] = nc.dram_tensor("out", [BPC, SP, NW], BF16, kind="ExternalOutput").ap()

    with tile.TileContext(nc) as tc:
        _emit(tc, d)
    nc.compile()
    _CACHE["nc"] = nc
    return nc


def prep_inputs(inputs):
    """Host-side constant packing + state transpose. Returns dict of np arrays
    shared across cores (stateT is full-batch; shard before dispatch)."""
    state = np.asarray(inputs["state"], np.float32)
    head_w = np.asarray(inputs["head_w"], np.float32)
    head_b = np.asarray(inputs["head_b"], np.float32)
    tail_w = np.asarray(inputs["tail_w"], np.float32)
    tail_b = np.asarray(inputs["tail_b"], np.float32)
    U = np.asarray(inputs["U"], np.float32)
    cls_w = np.asarray(inputs["cls_w"], np.float32)

    # stateT paired pack: [B/2, 128, (kt, b01, y)], y zero-padded to 256
    stateT = np.zeros((B, H, SP), np.float32)
    stateT[:, :, :S] = state.transpose(0, 2, 1)
    stateT = stateT.reshape(B // 2, 2, KT, 128, SP).transpose(0, 3, 2, 1, 4)
    stateT = np.ascontiguousarray(
        stateT.reshape(B // 2, 128, KT * SP2).astype(NPBF)
    )

    hw_sb = np.zeros((128, KT, BSE), np.float32)
    hw_sb[:, :, :BS] = head_w.reshape(KT, 128, BS).transpose(1, 0, 2)
    hw_sb = np.ascontiguousarray(hw_sb.reshape(128, KT * BSE).astype(NPBF))
    tw_sb = np.zeros((128, KT, BSE), np.float32)
    tw_sb[:, :, :BS] = tail_w.reshape(KT, 128, BS).transpose(1, 0, 2)
    tw_sb = np.ascontiguousarray(tw_sb.reshape(128, KT * BSE).astype(NPBF))

    # ut blocks + 2 spare cols
    ut = np.zeros((BSE, O * BSE + 2), np.float32)
    blocks = ut[:, :O * BSE].reshape(BSE, O, BSE)
    blocks[:BS, :, :BS] = U.transpose(2, 0, 1)           # [j, o, i] = U[o,i,j]
    blocks[:, :, BS] = cls_w[:, BS + 1:2 * (BS + 1)].T   # Wt (incl ones coeff)
    # fold the Wh projection (A-term) into the ones-row of each block:
    # t1T row 120 is all-ones, so adding Wh_ext[o, i] here adds A[x, o]
    # (broadcast over y) to the final scores.
    blocks[BS, :, :] += cls_w[:, :BSE]
    ut = np.ascontiguousarray(ut.astype(NPBF))

    bias = np.zeros((BSE, 2), np.float32)
    bias[:BS, 0] = head_b
    bias[BS, 0] = 1.0                                    # ones-row constant
    bias[:BS, 1] = tail_b
    bias[BS, 1] = 1.0

    return {
        "stateT": stateT,
        "hw": hw_sb,
        "tw": tw_sb,
        "ut": ut,
        "bias": bias,
    }


def run(inputs, trace=False, trace_kwargs=None):
    nc = build_nc()
    full = prep_inputs(inputs)
    shared = {k: v for k, v in full.items() if k != "stateT"}
    in_maps = []
    for c in range(NCORES):
        m = dict(shared)
        m["stateT"] = np.ascontiguousarray(full["stateT"][c * NP:(c + 1) * NP])
        in_maps.append(m)
    res = run_bass_kernel_spmd(
        nc,
        in_maps,
        core_ids=list(range(NCORES)),
        trace=trace,
        **(trace_kwargs or {}),
    )
    out = np.concatenate([r["out"] for r in res.results], axis=0)
    # [B, 256, (o,y)] bf16 -> [B, S, S, O] f32, then + C on the host
    out = out.reshape(B, SP, O, SP).astype(np.float32)
    out = np.ascontiguousarray(out[:, :S, :, :S].transpose(0, 1, 3, 2))
    width_table = np.asarray(inputs["width_table"], np.float32)
    cls_w = np.asarray(inputs["cls_w"], np.float32)
    cls_b = np.asarray(inputs["cls_b"], np.float32)
    pos = np.arange(S)[None, :] - np.arange(S)[:, None] + 1
    pos = pos * (pos > 0)
    wproj = width_table @ cls_w[:, 2 * (BS + 1):].T + cls_b   # [256, 10]
    out += wproj[pos][None]
    return out, res


def kernel(**inputs):
    out, _ = run(inputs, trace=False)
    return out


if __name__ == "__main__":
    build_nc()
    print("build ok")


# revision 40
# speedup vs baseline: 1.0759x; 1.0568x over previous
"""Biaffine scorer kernel for Trainium2 (Bass/Tile), data-parallel over batch
across 8 NeuronCores.

Reference computation (per batch item b):
    h = leaky_relu(state @ head_w + head_b)          # (S, BS)
    t = leaky_relu(state @ tail_w + tail_b)          # (S, BS)
    scores1[x,y,o] = h[x] @ U[o] @ t[y]
    scores2[x,y,o] = Wh.h1[x] + Wt.t1[y] + Ww.wemb[x,y] + cls_b
    out = scores1 + scores2                          # (S, S, O)

Full-bf16 dataflow (tolerance 2e-2 rel; bf16 end-to-end measures ~5e-3):
the f32 baseline was DMA-bandwidth-bound, bf16 halves every byte moved.
The batch-independent width/cls_b table C = wproj[pos(x,y), o] is added
on the HOST, so every device-side PSUM evacuation is a single-pass copy,
split across ACT and DVE by a greedy busy-time balancer (GpSimd has no
PSUM port and 2-pass routes double-handle elements). Structure per core
(4 batch items as 2 pairs, 512 moving columns per matmul):

    warmup       ~26 dummy matmuls bridge the framework preamble to the
                 first state tile so the PE HAM throttle grants 2.4 GHz
                 before the projections (they gate the first output byte)
    state DMA    kt-pair quarters on the qAct ring; the kt-major
                 projection matmuls stream right behind the transfers
    ht1[p]       [121, 1024] = Prelu(w.T @ stateT + bias) (h | t), bias
                 and the ones-row feature enter via the ACT bias AP
    tUT[p]       [121, 2, 2560]: per o-pair, [U(o).T | folds] @ t1_bb;
                 contiguous psum, one strided (2,512) evac per group
    out[x,(o,y)] = h1T.T @ tUT in 512-col matmuls, [1024,1024,512]
                 chunks on a 4-deep [128,1024] PSUM pool; every chunk
                 DMAs out the moment its evac lands (qSP ring)

Pairs are software-pipelined: pair-1 proj/tut matmuls fill the PE while
pair-0 finals evacuations drain. The host packs constants, transposes
state, un-interleaves the (o,y)-major bf16 output and adds C in f32.
"""

import numpy as np
import ml_dtypes

import concourse.bass as bass
import concourse.bacc as bacc
import concourse.tile as tile
from concourse import mybir
from concourse.bass_utils import run_bass_kernel_spmd

# problem shape (hardcoded per harness contract)
B, S, H = 32, 255, 1024
BS, WD, O = 120, 20, 10
SP = 256            # padded S
SP2 = 2 * SP        # paired moving dim
NW = SP * O         # 2560
KT = H // 128       # 8
NCORES = 8
BPC = B // NCORES   # 4 batch items per core
NP = BPC // 2       # 2 pairs per core
BSE = BS + 1        # 121

F32 = mybir.dt.float32
BF16 = mybir.dt.bfloat16
NPBF = ml_dtypes.bfloat16

_CACHE: dict = {}


def _emit(tc, d):
    """Emit the per-core program. d: dict of DRAM APs."""
    from contextlib import ExitStack

    nc = tc.nc
    AF = mybir.ActivationFunctionType

    with ExitStack() as ctx:
        const = ctx.enter_context(tc.tile_pool(name="const", bufs=1))
        st_pool = ctx.enter_context(tc.tile_pool(name="st", bufs=2))
        ht_pool = ctx.enter_context(tc.tile_pool(name="ht", bufs=2))
        tut_pool = ctx.enter_context(tc.tile_pool(name="tut", bufs=2))
        out_pool = ctx.enter_context(tc.tile_pool(name="outp", bufs=4))
        # 4-deep [128,1024] PSUM pool: 1024-wide evacs amortize the
        # per-op access latency; depth 4 keeps the PE ahead of the evacs
        pp = ctx.enter_context(tc.tile_pool(name="pp", bufs=4, space="PSUM"))

        # greedy 2-engine evac balancer: C is added on the HOST, so all
        # PSUM evacuations are 1-pass copies split across ACT and DVE.
        busy = {"A": 0.0, "D": 0.0}

        def _act(w):
            return (w + 206) / 1.2

        def _dve(w):
            return (w + 145) / 0.96

        def evac_copy(dst, src, w):
            if busy["A"] + _act(w) <= busy["D"] + _dve(w):
                busy["A"] += _act(w)
                nc.scalar.activation(dst, src, AF.Copy)
            else:
                busy["D"] += _dve(w)
                nc.vector.tensor_copy(dst, src)

        # ---- persistent constants (qSP ring, in first-use order) ----
        # weights carry an extra zero column -> psum row 120 = 0; the ACT
        # bias AP then sets row 120 to Prelu(0 + 1.0) = 1.0 (the ones row).
        sb_hw = const.tile([128, KT * BSE], BF16)
        nc.sync.dma_start(sb_hw[:], d["hw"])
        sb_tw = const.tile([128, KT * BSE], BF16)
        nc.sync.dma_start(sb_tw[:], d["tw"])
        # bias: col 0 = head_b (+1.0 at row 120), col 1 = tail_b
        sb_bias = const.tile([BSE, 2], F32)
        nc.sync.dma_start(sb_bias[:], d["bias"])
        # ut: per-o [121, 121] blocks (U.T with Wt in col 120, Wh folded
        # into the ones-row), then 2 spare cols.
        sb_ut = const.tile([BSE, O * BSE + 2], BF16)
        nc.sync.dma_start(sb_ut[:], d["ut"])
        # ---- state loads (qAct ring): kt-pair quarters so the kt-major
        # projection matmuls stream right behind the DMA ----
        qtr = KT * SP2 // 4
        sb_st = []
        for p in range(NP):
            quarters = []
            for k in range(4):
                t = st_pool.tile([128, qtr], BF16, name=f"sT{k}")
                nc.scalar.dma_start(t[:], d["stateT"][p][:, k * qtr:(k + 1) * qtr])
                quarters.append(t)
            sb_st.append(quarters)

        # ---- PE warmup: dummy matmuls bridge preamble -> first state
        # quarter (~2.5us) so the HAM throttle grants 2.4 GHz before the
        # projections, which gate the first output byte.
        warm = const.tile([128, 128], BF16)
        nc.vector.memset(warm[:], 0.0)
        ps_w = pp.tile([128, 1024], F32, name="ps")
        for i in range(26):
            nc.tensor.matmul(
                ps_w[:, 0:128], lhsT=warm[:], rhs=warm[:], start=True, stop=True
            )

        ht1 = [None, None]   # [121, 1024] bf16: h1T cols 0:512, t1T 512:1024
        tUT = [None, None]   # [121, 2, 2560] bf16 per pair
        for p in range(NP):
            tUT[p] = tut_pool.tile([BSE, 2, NW], BF16, name="tUT")

        def proj(p):
            # head/tail projections -> ht1[p], bf16, via Prelu+bias evac.
            # kt-major order: each state quarter feeds both sides' matmuls
            # as soon as it lands (proj is DMA-paced, not PE-paced).
            ps_ht = pp.tile([128, 1024], F32, name="ps")
            ps_h = ps_ht[0:BSE, 0:512]
            ps_t = ps_ht[0:BSE, 512:1024]
            for kt in range(KT):
                st = sb_st[p][kt // 2]
                rhs = st[:, (kt % 2) * SP2:(kt % 2 + 1) * SP2]
                for ps, w in ((ps_h, sb_hw), (ps_t, sb_tw)):
                    nc.tensor.matmul(
                        ps[:],
                        lhsT=w[:, kt * BSE:(kt + 1) * BSE],
                        rhs=rhs,
                        start=(kt == 0),
                        stop=(kt == KT - 1),
                    )
            ht1[p] = ht_pool.tile([BSE, 2 * SP2], BF16, name="ht1")
            nc.scalar.activation(
                ht1[p][:, 0:SP2], ps_h[:], AF.Prelu,
                bias=sb_bias[:, 0:1], alpha=0.01,
            )
            nc.scalar.activation(
                ht1[p][:, SP2:2 * SP2], ps_t[:], AF.Prelu,
                bias=sb_bias[:, 1:2], alpha=0.01,
            )
            busy["A"] += 2 * _act(512)

        def tut_group(p, q):
            # one 1024-wide PSUM tile = o-pair (2q, 2q+1) for both b0/b1;
            # contiguous psum src, one strided (2,512)-dst evac writes
            # chunk q*512 of both tUT[p][:, bb, :] rows.
            t1T = ht1[p][:, SP2:2 * SP2]
            ps_u = pp.tile([128, 1024], F32, name="ps")[0:BSE]
            for bb in range(2):
                rhs = t1T[:, bb * SP:(bb + 1) * SP]
                for s in range(2):
                    nc.tensor.matmul(
                        ps_u[:, (2 * bb + s) * SP:(2 * bb + s + 1) * SP],
                        lhsT=sb_ut[:, (2 * q + s) * BSE:(2 * q + s + 1) * BSE],
                        rhs=rhs,
                        start=True,
                        stop=True,
                    )
            # contiguous psum src; strided (2,512) dst into both tUT rows
            evac_copy(tUT[p][:, :, q * 512:(q + 1) * 512], ps_u[:], 1024)

        CHUNKS = ((0, 1024), (1024, 1024), (2048, 512))
        sbouts = {}

        def finals_chunk(p, bb, xt, ci):
            # one [128, w] chunk of out[x, (o,y)]; DMAs once its copy lands.
            key = (p, bb, xt)
            if key not in sbouts:
                sbouts[key] = out_pool.tile([128, NW], BF16, name="sb_out")
            sb_out = sbouts[key]
            lo = bb * SP + xt * 128
            lhsT = ht1[p][:, lo:lo + 128]
            c0, w = CHUNKS[ci]
            ps_s = pp.tile([128, 1024], F32, name="ps")
            for s in range(w // 512):
                nc.tensor.matmul(
                    ps_s[:, s * 512:(s + 1) * 512],
                    lhsT=lhsT,
                    rhs=tUT[p][:, bb, c0 + s * 512:c0 + (s + 1) * 512],
                    start=True,
                    stop=True,
                )
            evac_copy(sb_out[:, c0:c0 + w], ps_s[:, 0:w], w)
            nc.sync.dma_start(
                d["out"][2 * p + bb, xt * 128:xt * 128 + 128, c0:c0 + w],
                sb_out[:, c0:c0 + w],
            )

        def finals_tile(p, bb, xt):
            for ci in range(3):
                finals_chunk(p, bb, xt, ci)

        # emit order: first finals tile right after B0; proj(1)/tut(1)
        # matmuls fill PE while pair-0 finals evacs drain.
        proj(0)
        for q in range(5):
            tut_group(0, q)
        finals_tile(0, 0, 0)
        finals_tile(0, 0, 1)
        proj(1)
        finals_tile(0, 1, 0)
        tut_group(1, 0)
        finals_tile(0, 1, 1)
        tut_group(1, 1)
        tut_group(1, 2)
        tut_group(1, 3)
        tut_group(1, 4)
        finals_tile(1, 0, 0)
        finals_tile(1, 0, 1)
        finals_tile(1, 1, 0)
        finals_tile(1, 1, 1)


def build_nc():
    if "nc" in _CACHE:
        return _CACHE["nc"]
    nc = bacc.Bacc(
        "TRN2", target_bir_lowering=False, debug=False, num_devices=NCORES
    )
    d = {}
    d["stateT"] = nc.dram_tensor(
        "stateT", [NP, 128, KT * SP2], BF16, kind="ExternalInput"
    ).ap()
    d["hw"] = nc.dram_tensor("hw", [128, KT * BSE], BF16, kind="ExternalInput").ap()
    d["tw"] = nc.dram_tensor("tw", [128, KT * BSE], BF16, kind="ExternalInput").ap()
    d["ut"] = nc.dram_tensor(
        "ut", [BSE, O * BSE + 2], BF16, kind="ExternalInput"
    ).ap()
    d["bias"] = nc.dram_tensor("bias", [BSE, 2], F32, kind="ExternalInput").ap()
    d["out"] = nc.dram_tensor("out", [BPC, SP, NW], BF16, kind="ExternalOutput").ap()

    with tile.TileContext(nc) as tc:
        _emit(tc, d)
    nc.compile()
    _CACHE["nc"] = nc
    return nc


def prep_inputs(inputs):
    """Host-side constant packing + state transpose. Returns dict of np arrays
    shared across cores (stateT is full-batch; shard before dispatch)."""
    state = np.asarray(inputs["state"], np.float32)
    head_w = np.asarray(inputs["head_w"], np.float32)
    head_b = np.asarray(inputs["head_b"], np.float32)
    tail_w = np.asarray(inputs["tail_w"], np.float32)
    tail_b = np.asarray(inputs["tail_b"], np.float32)
    U = np.asarray(inputs["U"], np.float32)
    cls_w = np.asarray(inputs["cls_w"], np.float32)

    # stateT paired pack: [B/2, 128, (kt, b01, y)], y zero-padded to 256
    stateT = np.zeros((B, H, SP), np.float32)
    stateT[:, :, :S] = state.transpose(0, 2, 1)
    stateT = stateT.reshape(B // 2, 2, KT, 128, SP).transpose(0, 3, 2, 1, 4)
    stateT = np.ascontiguousarray(
        stateT.reshape(B // 2, 128, KT * SP2).astype(NPBF)
    )

    hw_sb = np.zeros((128, KT, BSE), np.float32)
    hw_sb[:, :, :BS] = head_w.reshape(KT, 128, BS).transpose(1, 0, 2)
    hw_sb = np.ascontiguousarray(hw_sb.reshape(128, KT * BSE).astype(NPBF))
    tw_sb = np.zeros((128, KT, BSE), np.float32)
    tw_sb[:, :, :BS] = tail_w.reshape(KT, 128, BS).transpose(1, 0, 2)
    tw_sb = np.ascontiguousarray(tw_sb.reshape(128, KT * BSE).astype(NPBF))

    # ut blocks + 2 spare cols
    ut = np.zeros((BSE, O * BSE + 2), np.float32)
    blocks = ut[:, :O * BSE].reshape(BSE, O, BSE)
    blocks[:BS, :, :BS] = U.transpose(2, 0, 1)           # [j, o, i] = U[o,i,j]
    blocks[:, :, BS] = cls_w[:, BS + 1:2 * (BS + 1)].T   # Wt (incl ones coeff)
    # fold the Wh projection (A-term) into the ones-row of each block:
    # t1T row 120 is all-ones, so adding Wh_ext[o, i] here adds A[x, o]
    # (broadcast over y) to the final scores.
    blocks[BS, :, :] += cls_w[:, :BSE]
    ut = np.ascontiguousarray(ut.astype(NPBF))

    bias = np.zeros((BSE, 2), np.float32)
    bias[:BS, 0] = head_b
    bias[BS, 0] = 1.0                                    # ones-row constant
    bias[:BS, 1] = tail_b
    bias[BS, 1] = 1.0

    return {
        "stateT": stateT,
        "hw": hw_sb,
        "tw": tw_sb,
        "ut": ut,
        "bias": bias,
    }


def run(inputs, trace=False, trace_kwargs=None):
    nc = build_nc()
    full = prep_inputs(inputs)
    shared = {k: v for k, v in full.items() if k != "stateT"}
    in_maps = []
    for c in range(NCORES):
        m = dict(shared)
        m["stateT"] = np.ascontiguousarray(full["stateT"][c * NP:(c + 1) * NP])
        in_maps.append(m)
    res = run_bass_kernel_spmd(
        nc,
        in_maps,
        core_ids=list(range(NCORES)),
        trace=trace,
        **(trace_kwargs or {}),
    )
    out = np.concatenate([r["out"] for r in res.results], axis=0)
    # [B, 256, (o,y)] bf16 -> [B, S, S, O] f32, then + C on the host
    out = out.reshape(B, SP, O, SP).astype(np.float32)
    out = np.ascontiguousarray(out[:, :S, :, :S].transpose(0, 1, 3, 2))
    width_table = np.asarray(inputs["width_table"], np.float32)
    cls_w = np.asarray(inputs["cls_w"], np.float32)
    cls_b = np.asarray(inputs["cls_b"], np.float32)
    pos = np.arange(S)[None, :] - np.arange(S)[:, None] + 1
    pos = pos * (pos > 0)
    wproj = width_table @ cls_w[:, 2 * (BS + 1):].T + cls_b   # [256, 10]
    out += wproj[pos][None]
    return out, res


def kernel(**inputs):
    out, _ = run(inputs, trace=False)
    return out


if __name__ == "__main__":
    build_nc()
    print("build ok")
